# revision 7
# baseline (speedup 1.0000x reference)
"""Causal self-attention (B=2, S=2048, E=2048, H=16, D=128) with RoPE,
tensor-parallel over 8 TRN2 NeuronCores (2 heads per core).

Strategy:
- Host: transpose x -> xT [E, B*S]; slice Wqkv/Wout per core; precompute
  RoPE cos/sin (transposed), the rotate-half permutation matrix, causal
  boundary masks, and ones vectors.
- Device (per core, SPMD): QKV projection with weights stationary gives
  q,k in [D, S] layout; V with x stationary gives vT in [S, D] layout.
  Scores are computed transposed ([sk, sq] = k_chunk^T . q_block) so the
  exp'd probabilities feed the P.V matmul directly as the moving operand
  (no transposes anywhere). Softmax denominator via ones-vector matmul
  over the accumulated exp tiles; normalization via a rank-1 broadcast
  matmul. Causal masking: above-diagonal 128x512 chunks are skipped,
  boundary chunks multiplied by precomputed binary masks. exp() needs no
  max-subtraction (scores ~ N(0,1) for this problem's scale).
- All matmuls run in float32r (full PE rate, ~1e-4 relative error).
- Each core emits a partial [E, B*S] output (its 2 heads through its
  Wout column-slice); the host sums the 8 partials and transposes back.
"""

import numpy as np

import concourse.bass as bass
import concourse.bacc as bacc
import concourse.tile as tile
import concourse.mybir as mybir
from concourse import bass_utils

B, S, E, H = 2, 2048, 2048, 16
D = E // H  # 128
NCORES = 8
HPC = H // NCORES  # heads per core = 2
T = B * S  # 4096 tokens
ROPE_BASE = 10000.0
P = 128
TT = 512  # token tile (free dim of most matmuls)
NTT = S // TT  # token tiles per batch = 4
NC_E = E // P  # contraction chunks over E = 16
FQKV = 3 * HPC * D  # per-core qkv features = 768
SCALE = float(D) ** -0.5

f32 = mybir.dt.float32
f32r = mybir.dt.float32r
EXP = mybir.ActivationFunctionType.Exp


def _build_kernel(nc, tc, aps, phases="ABC"):
    xT, wqkv, wout, cosT, sinT, masks, pt, o128, o1, outT = aps

    import contextlib
    ctx = contextlib.ExitStack()
    with ctx:
        ctx.enter_context(nc.allow_low_precision(
            reason="fp32r (tf32-like) matmul operands are intentional"))
        const = ctx.enter_context(tc.tile_pool(name="const", bufs=1))
        sb = ctx.enter_context(tc.tile_pool(name="sb", bufs=2))
        ps = ctx.enter_context(tc.tile_pool(name="ps", bufs=1, space="PSUM"))

        # --- resident constants -------------------------------------------
        wq_all = const.tile([P, NC_E, FQKV], f32r)  # 48KB/part
        for c in range(NC_E):
            nc.sync.dma_start(wq_all[:, c, :], wqkv[c * P:(c + 1) * P, :])
        wout_sb = const.tile([P, HPC, E], f32r)  # 16KB/part
        for hl in range(HPC):
            nc.sync.dma_start(wout_sb[:, hl, :], wout[hl * P:(hl + 1) * P, :])
        cos_sb = const.tile([P, S], f32)
        sin_sb = const.tile([P, S], f32)
        nc.sync.dma_start(cos_sb[:], cosT)
        nc.sync.dma_start(sin_sb[:], sinT)
        mask_sb = const.tile([P, 4, TT], mybir.dt.bfloat16)
        nc.sync.dma_start(mask_sb[:], masks.rearrange("p (r f) -> p r f", r=4))
        pt_sb = const.tile([P, P], f32r)
        nc.sync.dma_start(pt_sb[:], pt)
        o128_sb = const.tile([P, 1], f32r)
        nc.sync.dma_start(o128_sb[:], o128)
        o1_sb = const.tile([1, P], f32r)
        nc.sync.dma_start(o1_sb[:], o1)

        qr_t = {}  # (b, hl) -> [128, S] f32r   q after rope, [d, s]
        kr_t = {}
        vt_t = {}  # b -> [128, S//P, HPC*D]    vT tiles, [s%128, s//128, hd]
        ctx_t = {}  # (b, hl) -> [128, S] f32r  normalized context, [d, s]

        import os
        nbatch = int(os.environ.get("K_NBATCH", B))
        for b in range(nbatch):
            # ======== phase A: qkv projection + rope for batch b ==========
            vt_b = sb.tile([P, S // P, HPC * D], f32r, tag="vt", bufs=1)
            vt_t[b] = vt_b
            for hl in range(HPC):
                qr_t[(b, hl)] = sb.tile([P, S], f32r, tag="qr", bufs=HPC, name=f"qr_{b}_{hl}")
                kr_t[(b, hl)] = sb.tile([P, S], f32r, tag="kr", bufs=HPC, name=f"kr_{b}_{hl}")
            for tt in range(NTT):
                col0 = b * S + tt * TT
                xs = []
                for c in range(NC_E):
                    xt = sb.tile([P, TT], f32r, tag="x", bufs=17)
                    nc.sync.dma_start(xt[:], xT[c * P:(c + 1) * P, col0:col0 + TT])
                    xs.append(xt)
                # q,k blocks: [feature, token] layout; rope applied
                for fb in range(2 * HPC):  # q_h0, q_h1, k_h0, k_h1
                    is_q = fb < HPC
                    hl = fb % HPC
                    pmm = ps.tile([P, TT], f32, tag="mm", bufs=3)
                    for c in range(NC_E):
                        nc.tensor.matmul(
                            pmm[:], wq_all[:, c, fb * P:(fb + 1) * P], xs[c][:],
                            start=(c == 0), stop=(c == NC_E - 1))
                    raw = sb.tile([P, TT], f32r, tag="qraw", bufs=2)
                    nc.vector.tensor_copy(raw[:], pmm[:])
                    prot = ps.tile([P, TT], f32, tag="rot", bufs=1)
                    nc.tensor.matmul(prot[:], pt_sb[:], raw[:],
                                     start=True, stop=True)
                    t1 = sb.tile([P, TT], f32, tag="rt", bufs=1)
                    cs = slice(tt * TT, (tt + 1) * TT)
                    nc.vector.tensor_mul(t1[:], raw[:], cos_sb[:, cs])
                    dst = (qr_t if is_q else kr_t)[(b, hl)][:, cs]
                    nc.vector.tensor_mul(dst, prot[:], sin_sb[:, cs])
                    nc.vector.tensor_add(dst, dst, t1[:])
                # v: [token, vfeature] layout (both heads side by side)
                for sub in range(TT // P):
                    pv = ps.tile([P, TT], f32, tag="mm", bufs=3)
                    for c in range(NC_E):
                        nc.tensor.matmul(
                            pv[:, :HPC * D],
                            xs[c][:, sub * P:(sub + 1) * P],
                            wq_all[:, c, 2 * HPC * P:],
                            start=(c == 0), stop=(c == NC_E - 1))
                    nc.vector.tensor_copy(vt_b[:, tt * (TT // P) + sub, :],
                                          pv[:, :HPC * D])

            # ======== phase B: attention for batch b ======================
            for hl in range(HPC if "B" in phases else 0):
                ctile = sb.tile([P, S], f32r, tag="ctx", bufs=2)
                ctx_t[(b, hl)] = ctile
                qr = qr_t[(b, hl)]
                kr = kr_t[(b, hl)]
                for j in range(NTT):  # sq block of 512
                    nchunks = 4 * j + 4
                    pctx = ps.tile([P, TT], f32, tag="ctxp", bufs=1)
                    acc = sb.tile([P, TT], f32r, tag="accT", bufs=2)
                    qs = slice(j * TT, (j + 1) * TT)
                    for c in range(nchunks):
                        psc = ps.tile([P, TT], f32, tag="sc", bufs=2)
                        nc.tensor.matmul(psc[:], kr[:, c * P:(c + 1) * P],
                                         qr[:, qs], start=True, stop=True)
                        ex = sb.tile([P, TT], f32r, tag="expT", bufs=3)
                        nc.scalar.activation(ex[:], psc[:], EXP, scale=SCALE)
                        r = c - 4 * j
                        if r >= 0:  # boundary chunk: causal binary mask
                            nc.vector.tensor_mul(ex[:], ex[:], mask_sb[:, r, :])
                        nc.tensor.matmul(pctx[:], vt_t[b][:, c, hl * D:(hl + 1) * D],
                                         ex[:], start=(c == 0),
                                         stop=(c == nchunks - 1))
                        if c == 0:
                            nc.vector.tensor_copy(acc[:], ex[:])
                        else:
                            nc.vector.tensor_add(acc[:], acc[:], ex[:])
                    lp = ps.tile([P, TT], f32, tag="sm", bufs=1)
                    nc.tensor.matmul(lp[0:1, :], o128_sb[:], acc[:],
                                     start=True, stop=True)
                    linv = sb.tile([1, TT], f32r, tag="linv", bufs=1)
                    nc.vector.reciprocal(linv[:], lp[0:1, :])
                    bp = ps.tile([P, TT], f32, tag="sm", bufs=1)
                    nc.tensor.matmul(bp[:], o1_sb[:], linv[:],
                                     start=True, stop=True)
                    lb = sb.tile([P, TT], f32, tag="linvb", bufs=1)
                    nc.vector.tensor_copy(lb[:], bp[:])
                    nc.vector.tensor_mul(ctile[:, qs], pctx[:], lb[:])

            # ======== phase C: output projection for batch b ==============
            for j in range(NTT if "C" in phases else 0):
                qs = slice(j * TT, (j + 1) * TT)
                for of in range(E // P):
                    po = ps.tile([P, TT], f32, tag="mm", bufs=3)
                    for hl in range(HPC):
                        nc.tensor.matmul(
                            po[:], wout_sb[:, hl, of * P:(of + 1) * P],
                            ctx_t[(b, hl)][:, qs],
                            start=(hl == 0), stop=(hl == HPC - 1))
                    ost = sb.tile([P, TT], f32, tag="ostage", bufs=2)
                    nc.vector.tensor_copy(ost[:], po[:])
                    nc.sync.dma_start(
                        outT[of * P:(of + 1) * P, b * S + j * TT:
                             b * S + (j + 1) * TT], ost[:])


def build_nc():
    nc = bacc.Bacc("TRN2", target_bir_lowering=False, debug=False,
                   num_devices=NCORES)
    xT = nc.dram_tensor("xT", [E, T], f32r, kind="ExternalInput").ap()
    wqkv = nc.dram_tensor("wqkvT", [E, FQKV], f32r, kind="ExternalInput").ap()
    wout = nc.dram_tensor("woutT", [HPC * D, E], f32r,
                          kind="ExternalInput").ap()
    cosT = nc.dram_tensor("cosT", [D, S], f32, kind="ExternalInput").ap()
    sinT = nc.dram_tensor("sinT", [D, S], f32, kind="ExternalInput").ap()
    masks = nc.dram_tensor("masks", [P, 4 * TT], mybir.dt.bfloat16,
                           kind="ExternalInput").ap()
    pt = nc.dram_tensor("pt", [P, P], f32r, kind="ExternalInput").ap()
    o128 = nc.dram_tensor("o128", [P, 1], f32r, kind="ExternalInput").ap()
    o1 = nc.dram_tensor("o1", [1, P], f32r, kind="ExternalInput").ap()
    outT = nc.dram_tensor("outT", [E, T], f32, kind="ExternalOutput").ap()
    import os
    with tile.TileContext(nc) as tc:
        _build_kernel(nc, tc, (xT, wqkv, wout, cosT, sinT, masks, pt,
                               o128, o1, outT),
                      phases=os.environ.get("K_PHASES", "ABC"))
    nc.compile()
    return nc


def host_inputs(x, Wqkv, Wout):
    """Per-core input dicts (numpy, all fp32 bits)."""
    xT = np.ascontiguousarray(x.reshape(T, E).T).astype(np.float32)

    inv_freq = 1.0 / (ROPE_BASE ** (np.arange(0, D, 2, dtype=np.float64) / D))
    pos = np.arange(S, dtype=np.float64)
    freqs = np.outer(pos, inv_freq)            # [S, D/2]
    ang = np.concatenate([freqs, freqs], -1)   # [S, D]
    cosT = np.ascontiguousarray(np.cos(ang).T).astype(np.float32)
    sinT = np.ascontiguousarray(np.sin(ang).T).astype(np.float32)

    ptm = np.zeros((P, P), np.float32)  # rh(q) = ptm.T @ q
    for d in range(0, P, 2):
        ptm[d + 1, d] = -1.0   # rh[even d] = -q[d+1]
        ptm[d, d + 1] = 1.0    # rh[odd d]  =  q[d-1]

    masks = np.zeros((P, 4 * TT), np.float32)
    pp = np.arange(P)[:, None]
    ff = np.arange(TT)[None, :]
    for r in range(4):
        masks[:, r * TT:(r + 1) * TT] = (pp <= ff - 128 * r).astype(np.float32)

    import ml_dtypes
    masks_bf16 = masks.astype(ml_dtypes.bfloat16)
    o128 = np.ones((P, 1), np.float32)
    o1 = np.ones((1, P), np.float32)

    in_maps = []
    for c in range(NCORES):
        r0 = HPC * D * c  # 256*c
        wq = Wqkv[r0:r0 + HPC * D]
        wk = Wqkv[E + r0:E + r0 + HPC * D]
        wv = Wqkv[2 * E + r0:2 * E + r0 + HPC * D]
        wqkvT = np.ascontiguousarray(
            np.concatenate([wq, wk, wv], 0).T).astype(np.float32)
        woutT = np.ascontiguousarray(
            Wout[:, r0:r0 + HPC * D].T).astype(np.float32)
        in_maps.append({
            "xT": xT, "wqkvT": wqkvT, "woutT": woutT,
            "cosT": cosT, "sinT": sinT, "masks": masks_bf16, "pt": ptm,
            "o128": o128, "o1": o1,
        })
    return in_maps


_NC_CACHE = None


def kernel(x, Wqkv, Wout):
    global _NC_CACHE
    x = np.asarray(x)
    Wqkv = np.asarray(Wqkv)
    Wout = np.asarray(Wout)
    in_maps = host_inputs(x, Wqkv, Wout)
    if _NC_CACHE is None:
        _NC_CACHE = build_nc()
    res = bass_utils.run_bass_kernel_spmd(
        _NC_CACHE, in_maps, core_ids=list(range(NCORES)))
    acc = np.zeros((E, T), np.float64)
    for c in range(NCORES):
        acc += res.results[c]["outT"].astype(np.float64)
    out = acc.T.reshape(B, S, E).astype(np.float32)
    return out
